# revision 1
# baseline (speedup 1.0000x reference)
"""FUSE bucket-model scan kernel for Trainium2 (8 NeuronCores).

Strategy
--------
H=4096 HRUs are sharded across 8 cores (512 each). Each core holds its HRUs
as [128 partitions x 4 groups]; the two bucket states are packed with the
groups into [128, 8] tiles (cols 0-3: upper zone per group, cols 4-7: lower
zone per group). The T=8192 time recurrence runs as a sequential scan on
the device; all forcing-derived per-step operand tiles (Z = [p*inv1 |
baserte*inv2], A = [-(pet+percrte)*inv1 | percrte*inv2]) are precomputed on
the host in a scan-friendly [128, T, 8] layout and streamed in per K-step
chunk. The loop-carried chain per step (everything else runs in its shadow):

  r -> l = Ln(r) -> m = l*PW -> x = Exp(m) -> h = x*Z_t
    -> r' = clip(phi - h, eps, 1)   [custom fused DVE op]

with off-chain per step:
  w_l/w_r = r1 * A_t; phi = sigma + w (+pn on the left half)
  sigma' = clip(phi - h, 0, 1)       [custom fused DVE op]
  runoff = (h*MW) pair-summed into the chunk output tile

State is normalized (sigma = s/maxwatr in [0,1]) so both clips take
immediate bounds and fuse into single instructions. The scan is
latency-bound (~1.6us/step: two ACT transcendental round-trips + three
DVE nodes); streaming, runoff output, and the phi/w arithmetic all fit
inside that latency shadow (h/m and runoff ops on Pool keep the DVE
queue clear for the fused clips). Model-estimated device time ~12.3 ms;
output matches the jax reference to ~8e-6 relative-of-max.
"""
import numpy as np

import concourse.bass as bass
import concourse.bacc as bacc
import concourse.mybir as mybir
from concourse.bass import ds
from concourse.tile import TileContext
from concourse.bass_utils import run_bass_kernel_spmd

F32 = mybir.dt.float32
AF = mybir.ActivationFunctionType
OP = mybir.AluOpType


# --- custom fused DVE ops ---------------------------------------------------
# The loop-carried chain is sigma/r -> Ln -> mul -> Exp -> h -> next state.
# Fusing "u = phi - h; sigma' = clip(u,0,1)" and "r' = clip(u,eps,1)" into one
# custom DVE instruction each removes two DVE nodes from that chain (r' feeds
# the next Ln directly; sigma' is consumed off-chain by the phi ops).
from concourse.dve_spec import Spec, Src0, Src1, maxx, minn, lower as _dve_lower
from concourse.dve_spec import C0 as _C0, One as _One, Zero as _Zero
from concourse import dve_ops as _dvo
from concourse.dve_uop import DveOpSpec as _DveOpSpec


def _register_custom_op(name, spec):
    for op in _dvo.OPS:
        if op.name == name:
            return op
    row = _dvo._CUSTOM_DVE_ROW_BASE + len(_dvo.OPS)
    _dvo._SUB_OPCODE_FOR_NAME[name] = row
    shas = {}
    for ver in ("v3", "v4"):
        try:
            uops = _dve_lower(spec, ver=ver)
            shas[ver] = _DveOpSpec(name=name, opcode=row, uops=uops,
                                   rd1_en=True).sha(ver)
        except Exception:
            pass
    op = _dvo.DveOp(name, spec, subdim=False, uops_sha=shas)
    _dvo.OPS.append(op)
    _dvo.CUSTOM_DVE_SPECS[name] = spec
    return op


SIG_CLIPSUB = _register_custom_op(
    "FUSE_SIG_CLIPSUB",
    Spec(
        body=minn(maxx(Src0 - Src1, _Zero), _One),
        reference=lambda in0, in1, s0, s1, imm2: np.clip(
            (in0.astype(np.float32) - in1).astype(np.float32),
            np.float32(0.0), np.float32(1.0)),
    ),
)
R_CLIPSUB = _register_custom_op(
    "FUSE_R_CLIPSUB",
    Spec(
        body=minn(maxx(Src0 - Src1, _C0), _One),
        reference=lambda in0, in1, s0, s1, imm2: np.clip(
            (in0.astype(np.float32) - in1).astype(np.float32),
            np.float32(s0), np.float32(1.0)),
    ),
)

T = 8192
H = 4096
NCORES = 8
HC = H // NCORES          # 512 HRUs per core
P = 128                   # partitions
G = HC // P               # 4 groups
K = 128                   # timesteps per chunk
EPS = 1e-6


def build_nc(t_total=T, k_chunk=K, unrolled=False, ro_pool=True, w_pool=False, m_pool=True, w3d=True, prio=False, h_pool=True, zc2_pool=False, wbufs=4):
    nc = bacc.Bacc()
    ZAt = nc.dram_tensor("ZA", [P, t_total * 16], F32, kind="ExternalInput")
    Ct = nc.dram_tensor("CONSTS", [P, 24], F32, kind="ExternalInput")
    RO = nc.dram_tensor("RO", [G * P, t_total], F32, kind="ExternalOutput")

    n_chunks = t_total // k_chunk
    with TileContext(nc) as tc:
        with (
            tc.tile_pool(name="const", bufs=1) as cpool,
            tc.tile_pool(name="zin", bufs=3) as zpool,
            tc.tile_pool(name="rout", bufs=3) as ropool,
            tc.tile_pool(name="work", bufs=wbufs) as wpool,
        ):
            cst_in = cpool.tile([P, 24], F32)
            cst = cpool.tile([P, 24], F32)
            nc.sync.dma_start(out=cst_in[:], in_=Ct[:])
            # pre-loop copy: the loop body then only depends on DVE-written
            # tiles for its constants, keeping per-instruction wait lists small
            nc.vector.tensor_copy(out=cst[:], in_=cst_in[:])
            pw = cst[:, 8:16]
            mw = cst[:, 16:24]
            sigt = cpool.tile([P, 8], F32)
            rrt = cpool.tile([P, 8], F32)
            nc.vector.tensor_copy(out=sigt[:], in_=cst[:, 0:8])
            nc.vector.tensor_scalar(out=rrt[:], in0=cst[:, 0:8], scalar1=EPS,
                                    scalar2=None, op0=OP.max)
            sig = sigt[:]

            import contextlib
            def chunk_iter():
                if unrolled:
                    for i in range(n_chunks):
                        yield contextlib.nullcontext(i)
                else:
                    yield tc.For_i(0, n_chunks, staggered_reset=True,
                                   hint_engines=(mybir.EngineType.DVE,
                                                 mybir.EngineType.Activation))
            for _cm in chunk_iter():
              with _cm as ci:
                  zc = zpool.tile([P, k_chunk * 16], F32)
                  ro = ropool.tile([P, k_chunk * 4], F32)
                  nc.sync.dma_start(out=zc[:], in_=ZAt[:, ds(ci * (k_chunk * 16), k_chunk * 16)])
                  # DVE pre-touch: the DVE ISA struct allows only one semaphore
                  # wait per instruction, so absorb the DMA-completion wait in a
                  # dedicated copy; the scan ops then only wait on ACT.
                  zc2 = zpool.tile([P, k_chunk * 16], F32, tag="zc2")
                  (nc.gpsimd if zc2_pool else nc.vector).tensor_copy(out=zc2[:], in_=zc[:])

                  for k in range(k_chunk):
                      z8 = zc2[:, k * 16:k * 16 + 8]
                      a8 = zc2[:, k * 16 + 8:k * 16 + 16]
                      l = wpool.tile([P, 8], F32, tag="l")
                      m = wpool.tile([P, 8], F32, tag="m")
                      x = wpool.tile([P, 8], F32, tag="x")
                      h = wpool.tile([P, 8], F32, tag="h")
                      w = wpool.tile([P, 8], F32, tag="w")
                      phi = wpool.tile([P, 8], F32, tag="phi")
                      ra = wpool.tile([P, 8], F32, tag="ra")

                      i1 = nc.scalar.activation(l[:], rrt[:], AF.Ln)
                      meng = nc.gpsimd if m_pool else nc.vector
                      i2 = meng.tensor_tensor(out=m[:], in0=l[:], in1=pw, op=OP.mult)
                      i3 = nc.scalar.activation(x[:], m[:], AF.Exp)
                      heng = nc.gpsimd if h_pool else nc.vector
                      i4 = heng.tensor_tensor(out=h[:], in0=x[:], in1=z8, op=OP.mult)
                      if prio:
                          for bi in (i1, i2, i3, i4):
                              bi.ins.bass_priority = -100

                      weng = nc.gpsimd if w_pool else nc.vector
                      if w3d:
                          r1b = rrt[:, 0:4].rearrange('p (o f) -> p o f', o=1) \
                                           .broadcast_to([P, 2, 4])
                          a3 = a8.rearrange('p (o f) -> p o f', o=2)
                          w3 = w[:].rearrange('p (o f) -> p o f', o=2)
                          weng.tensor_tensor(out=w3, in0=r1b, in1=a3, op=OP.mult)
                      else:
                          weng.tensor_tensor(out=w[:, 0:4], in0=rrt[:, 0:4],
                                                  in1=a8[:, 0:4], op=OP.mult)
                          weng.tensor_tensor(out=w[:, 4:8], in0=rrt[:, 0:4],
                                                  in1=a8[:, 4:8], op=OP.mult)
                      weng.tensor_tensor(out=phi[:], in0=sig, in1=w[:], op=OP.add)
                      weng.tensor_tensor(out=phi[:, 0:4], in0=phi[:, 0:4],
                                              in1=z8[:, 0:4], op=OP.add)
                      # fused (phi - h) -> clip: state and next-r in one node each
                      nc.vector._custom_dve(SIG_CLIPSUB, out=sig, in0=phi[:], in1=h[:])
                      nc.vector._custom_dve(R_CLIPSUB, out=rrt[:], in0=phi[:],
                                            in1=h[:], s0=EPS)

                      roeng = nc.gpsimd if ro_pool else nc.vector
                      roeng.tensor_tensor(out=ra[:], in0=h[:], in1=mw, op=OP.mult)
                      rocol = ro[:].rearrange('p (g t) -> p g t', g=G)[:, :, k]
                      roeng.tensor_tensor(out=rocol,
                                              in0=ra[:, 0:4], in1=ra[:, 4:8], op=OP.add)

                  rov = ro[:].rearrange('p (g t) -> p g t', g=G)
                  dst = RO.rearrange('(g p) t -> p g t', g=G)[:, :, ds(ci * k_chunk, k_chunk)]
                  nc.sync.dma_start(out=dst, in_=rov)
    _compile_with_combined_ln_exp_table(nc)
    return nc


def _compile_with_combined_ln_exp_table(nc):
    """Bacc's act-table chooser picks separate `exp` and `ln` sets, inserting
    a ~1.3us table load before every activation (2.6us/step!). Both live in
    the `natural_log_exp_and_others` set; strip them from all other sets
    (keeping dict order, since the set id is positional) so the fixpoint
    resolves both to the combined set and hoists one load to the entry."""
    orig = bacc.get_activation_tables
    want = {mybir.ActivationFunctionType.Ln, mybir.ActivationFunctionType.Exp}

    def patched(arch):
        tabs = orig(arch)
        out = {}
        for name, funcs in tabs.items():
            if name != "natural_log_exp_and_others":
                funcs = funcs - want
            out[name] = funcs
        return out

    bacc.get_activation_tables = patched
    try:
        nc.compile()
    finally:
        bacc.get_activation_tables = orig


def _host_prepare(forcing, initial_state, raw_params, param_lower, param_upper,
                  t_total=T):
    """Derive per-core input arrays. All fp32, same op order as the sim."""
    f32 = np.float32
    lo = param_lower.astype(f32)
    hi = param_upper.astype(f32)
    # sigmoid in f64 then round: within 2ulp of jax.nn.sigmoid; end impact is nil
    sg = (1.0 / (1.0 + np.exp(-raw_params.astype(np.float64))))
    phys = (lo.astype(np.float64) + (hi - lo).astype(np.float64) * sg).astype(f32)
    mw1, mw2, percrte, baserte, qbp, axv = [phys[:, i].copy() for i in range(6)]
    inv1 = (f32(1.0) / mw1).astype(f32)
    inv2 = (f32(1.0) / mw2).astype(f32)

    p_r = forcing[:, :, 0].astype(f32)    # [T, H]
    pet = forcing[:, :, 1].astype(f32)

    pn = (p_r * inv1[None, :]).astype(f32)
    al = (-((pet + percrte[None, :]) * inv1[None, :])).astype(f32)
    bn = (baserte * inv2).astype(f32)
    pc12 = (percrte * inv2).astype(f32)

    s1n = (initial_state[:, 0].astype(f32) * inv1).astype(f32)
    s2n = (initial_state[:, 1].astype(f32) * inv2).astype(f32)

    in_maps = []
    for c in range(NCORES):
        sl = slice(c * HC, (c + 1) * HC)
        # [T, HC] -> [T, G, P] -> [P, T, G]
        def tg(a):
            return np.ascontiguousarray(
                a[:, sl].reshape(t_total, G, P).transpose(2, 0, 1))
        pn_c = tg(pn)          # [P, T, G]
        al_c = tg(al)
        ZA = np.empty((P, t_total, 16), f32)
        ZA[:, :, 0:4] = pn_c
        ZA[:, :, 4:8] = bn[sl].reshape(G, P).T[:, None, :]
        ZA[:, :, 8:12] = al_c
        ZA[:, :, 12:16] = pc12[sl].reshape(G, P).T[:, None, :]

        def pk(a1, a2):
            out = np.empty((P, 8), f32)
            out[:, 0:4] = a1[sl].reshape(G, P).T
            out[:, 4:8] = a2[sl].reshape(G, P).T
            return out

        consts = np.concatenate([pk(s1n, s2n), pk(axv, qbp), pk(mw1, mw2)],
                                axis=1)
        in_maps.append({
            "ZA": ZA.reshape(P, t_total * 16),
            "CONSTS": consts,
        })
    return in_maps


_NC_CACHE = {}


def kernel(forcing, initial_state, raw_params, param_lower, param_upper):
    forcing = np.asarray(forcing)
    initial_state = np.asarray(initial_state)
    raw_params = np.asarray(raw_params)
    param_lower = np.asarray(param_lower)
    param_upper = np.asarray(param_upper)
    t_total = forcing.shape[0]
    if t_total not in _NC_CACHE:
        _NC_CACHE[t_total] = build_nc(t_total=t_total)
    nc = _NC_CACHE[t_total]
    in_maps = _host_prepare(forcing, initial_state, raw_params,
                            param_lower, param_upper, t_total=t_total)
    res = run_bass_kernel_spmd(nc, in_maps, core_ids=list(range(NCORES)))
    # per-core RO: [T, G, P] with h_local = g*P + p
    out = np.empty((t_total, H), np.float32)
    for c in range(NCORES):
        ro = res.results[c]["RO"]           # [HC, T], row = g*P + p
        out[:, c * HC:(c + 1) * HC] = ro.T
    return out



# revision 13
# speedup vs baseline: 1.7045x; 1.7045x over previous
"""FUSE bucket-model scan kernel for Trainium2 (8 NeuronCores) — all-DVE scan.

Strategy
--------
H=4096 HRUs sharded across 8 cores (512 each) as [128 partitions x 4 groups];
the two bucket states are packed into one [128, 8] tile (cols 0-3: upper zone
per group, cols 4-7: lower zone). The T=8192 time recurrence is a sequential
scan; all forcing-derived operand tiles (Z = [pn | bn], A' = [1-(pet+perc)/m1
| perc/m2]) are precomputed on the host in a scan-friendly [128, T, 16]
layout and streamed per K-step chunk.

The baseline ran the power nonlinearity (r^a = exp(a ln r)) through the
Activation engine: two transcendental round-trips + cross-engine hops gave a
~1.5us/step loop-carried chain. This kernel instead computes r^a entirely on
the Vector engine (DVE) with fused custom multi-stage ops, so every op in the
cycle sits on ONE engine queue and dependent ops chain back-to-back at
~70ns/instruction with no semaphore hops:

  LOG1/LOG2: log2(r) via IEEE-754 bit tricks — mu = bits(r)|bits(1.0) in
    [1,2) (1 ALU stage; valid for r<2), exponent from float(int32-view)*2^-23,
    deg-4 minimax poly for log2(mu);
  EXP1: y = log2(r)*a (per-element exponents), v = max(y/16, -1);
  EXP2/EXP3: 2^y = ((1 + v*P2(v))^16) * Z — constrained deg-3 poly + 4
    squarings, with the h = x*Z multiply folded into the last op;
  W/PHI: phi = [r1*A'1 + pn | r1*pc12 + r2] (stock tensor ops);
  RCLIP: r' = clip(phi - h, eps, 1) (fused clip-sub custom op).

9 DVE instructions/step total; runoff (h*MW pair-summed) and the chunk
stream pre-touch run on Pool in the latency shadow. State is normalized
r = clip(s/maxwatr, eps, 1); carrying r instead of (sigma, r) loses at most
eps/step, far under tolerance. End-to-end vs the f64 reference: rel-of-max
~3e-4 (tolerance 2e-2). Model-estimated device time ~5ms vs 12.3ms baseline.
"""
import numpy as np

import concourse.bass as bass
import concourse.bacc as bacc
import concourse.mybir as mybir
from concourse.bass import ds
from concourse.tile import TileContext
from concourse.bass_utils import run_bass_kernel_spmd

F32 = mybir.dt.float32
I32 = mybir.dt.int32
AF = mybir.ActivationFunctionType
OP = mybir.AluOpType

# --- custom fused DVE ops ---------------------------------------------------
from concourse.dve_spec import Spec, Src0, Src1, maxx, minn, lower as _dve_lower
from concourse.dve_spec import C0 as _C0, C1 as _C1, C2 as _C2, One as _One, Zero as _Zero
from concourse.dve_spec import Bin, AluOp
from concourse import dve_ops as _dvo
from concourse.dve_uop import DveOpSpec as _DveOpSpec


def _register_custom_op(name, spec):
    for op in _dvo.OPS:
        if op.name == name:
            return op
    row = _dvo._CUSTOM_DVE_ROW_BASE + len(_dvo.OPS)
    _dvo._SUB_OPCODE_FOR_NAME[name] = row
    shas = {}
    for ver in ("v3", "v4"):
        try:
            uops = _dve_lower(spec, ver=ver)
            shas[ver] = _DveOpSpec(name=name, opcode=row, uops=uops,
                                   rd1_en=True).sha(ver)
        except Exception:
            pass
    op = _dvo.DveOp(name, spec, subdim=False, uops_sha=shas)
    _dvo.OPS.append(op)
    _dvo.CUSTOM_DVE_SPECS[name] = spec
    return op


# Polynomial coefficients (Chebyshev LS fits).
# log2(mu) on [1,2], deg 4 (c0 folded into the LOG1 bias constant):
LC = [-2.496773767905599, 4.0283727668469735, -2.0810602034595114,
      0.6288157291849531, -0.07915036575317282]
# (2^v - 1)/v on [-1,0], deg 2  =>  2^v ~= 1 + v*(EC0 + v*(EC1 + v*EC2)):
EC = [0.6927658142758559, 0.23552592911390743, 0.043112826547017494]
C23 = float(2.0 ** -23)
CL1 = float(126.0 - LC[0])

_f = lambda x: np.asarray(x, np.float32)


def _mu_of(in1):
    return (np.asarray(in1, np.float32).view(np.int32)
            | np.int32(0x3F800000)).view(np.float32)


def _ref_log1(in0, in1, s0, s1, imm2):
    f = in0.astype(np.float32)
    y0 = _f(_f(f * np.float32(s0)) - np.float32(s1))
    mu = _mu_of(in1)
    e1 = _f(y0 - mu)
    mu2 = _f(mu * mu)
    mu4 = _f(mu2 * mu2)
    return _f(e1 + _f(mu4 * np.float32(imm2)))


_mu0 = Bin(AluOp.BITWISE_OR, Src1, _One)
_mu0sq = _mu0 * _mu0
FUSE_LOG1 = _register_custom_op(
    "FUSE_LOG1",
    Spec(body=((Src0 * _C0 - _C1) - _mu0) + (_mu0sq * _mu0sq) * _C2,
         reference=_ref_log1),
)


def _ref_log2(in0, in1, s0, s1, imm2):
    mu = _mu_of(in0)
    E = _f(_f(_f(_f(_f(mu * np.float32(s0)) + np.float32(s1)) * mu)
              + np.float32(imm2)) * mu)
    return _f(E + in1.astype(np.float32))


_mu1 = Bin(AluOp.BITWISE_OR, Src0, _One)
FUSE_LOG2 = _register_custom_op(
    "FUSE_LOG2",
    Spec(body=((_mu1 * _C0 + _C1) * _mu1 + _C2) * _mu1 + Src1,
         reference=_ref_log2),
)


def _ref_expa(in0, in1, s0, s1, imm2):
    # in0 = lam, in1 = PW/16 (the /16 range reduction is host-folded)
    y = _f(in0.astype(np.float32) * in1.astype(np.float32))
    v = np.maximum(y, np.float32(-1.0))
    q = _f(_f(_f(_f(_f(v * np.float32(imm2)) + np.float32(s1)) * v)
              + np.float32(s0)) * v)
    return _f(q + np.float32(1.0))


_v = maxx(Src0 * Src1, _Zero - _One)
_q = ((_v * _C2 + _C1) * _v + _C0) * _v
FUSE_EXPA = _register_custom_op(
    "FUSE_EXPA",
    Spec(body=_q + _One, reference=_ref_expa),
)


def _ref_exp3(in0, in1, s0, s1, imm2):
    u = in0.astype(np.float32)
    u2 = _f(u * u)
    u4 = _f(u2 * u2)
    u8 = _f(u4 * u4)
    u16 = _f(u8 * u8)
    return _f(u16 * in1.astype(np.float32))


_su2 = Src0 * Src0
_su4 = _su2 * _su2
_su8 = _su4 * _su4
_su16 = _su8 * _su8
FUSE_EXP3 = _register_custom_op(
    "FUSE_EXP3",
    Spec(body=_su16 * Src1, reference=_ref_exp3),
)


def _ref_clip(in0, in1, s0, s1, imm2):
    return np.clip(_f(in0.astype(np.float32) - in1.astype(np.float32)),
                   np.float32(s0), np.float32(1.0))


FUSE_RCLIP = _register_custom_op(
    "FUSE_RCLIP",
    Spec(body=minn(maxx(Src0 - Src1, _C0), _One), reference=_ref_clip),
)

T = 8192
H = 4096
NCORES = 8
HC = H // NCORES          # 512 HRUs per core
P = 128                   # partitions
G = HC // P               # 4 groups
K = 128                   # timesteps per chunk
EPS = 1e-6


def build_nc(t_total=T, k_chunk=K, unrolled=False):
    nc = bacc.Bacc()
    ZAt = nc.dram_tensor("ZA", [P, t_total * 16], F32, kind="ExternalInput")
    Ct = nc.dram_tensor("CONSTS", [P, 24], F32, kind="ExternalInput")
    # raw h = r^a * [pn | bn] streamed out; host folds maxwatr + pair-sum
    HO = nc.dram_tensor("HO", [P, t_total * 8], F32, kind="ExternalOutput")

    n_chunks = t_total // k_chunk
    with TileContext(nc) as tc:
        with (
            tc.tile_pool(name="const", bufs=1) as cpool,
            tc.tile_pool(name="zin", bufs=3) as zpool,
            tc.tile_pool(name="rout", bufs=3) as ropool,
            tc.tile_pool(name="work", bufs=4) as wpool,
        ):
            cst_in = cpool.tile([P, 24], F32)
            cst = cpool.tile([P, 24], F32)
            nc.sync.dma_start(out=cst_in[:], in_=Ct[:])
            nc.vector.tensor_copy(out=cst[:], in_=cst_in[:])
            pw = cst[:, 8:16]
            rrt = cpool.tile([P, 8], F32)
            # state: r = clip(s/maxwatr, eps, 1); all scan ops live on DVE
            nc.vector.tensor_scalar(out=rrt[:], in0=cst[:, 0:8], scalar1=EPS,
                                    scalar2=None, op0=OP.max)

            import contextlib
            def chunk_iter():
                if unrolled:
                    for i in range(n_chunks):
                        yield contextlib.nullcontext(i)
                else:
                    yield tc.For_i(0, n_chunks, staggered_reset=True,
                                   hint_engines=(mybir.EngineType.DVE,
                                                 mybir.EngineType.Pool))
            for _cm in chunk_iter():
              with _cm as ci:
                  zc = zpool.tile([P, k_chunk * 16], F32)
                  hc = ropool.tile([P, k_chunk * 8], F32)
                  nc.sync.dma_start(out=zc[:], in_=ZAt[:, ds(ci * (k_chunk * 16), k_chunk * 16)])
                  # pre-touch on Pool: absorbs the DMA-completion wait so DVE
                  # stream readers only ever wait on one Pool semaphore.
                  zc2 = zpool.tile([P, k_chunk * 16], F32, tag="zc2")
                  nc.gpsimd.tensor_copy(out=zc2[:], in_=zc[:])

                  for k in range(k_chunk):
                      z8 = zc2[:, k * 16:k * 16 + 8]
                      a8 = zc2[:, k * 16 + 8:k * 16 + 16]
                      e1t = wpool.tile([P, 8], F32, tag="e1t")
                      lam = wpool.tile([P, 8], F32, tag="lam")
                      u = wpool.tile([P, 8], F32, tag="u")
                      h = hc[:, k * 8:k * 8 + 8]
                      w = wpool.tile([P, 8], F32, tag="w")
                      phi = wpool.tile([P, 8], F32, tag="phi")

                      # log2(r) = e' + P4(mu), via int32-view exponent + OR-mantissa
                      nc.vector._custom_dve(FUSE_LOG1, out=e1t[:],
                                            in0=rrt[:].bitcast(I32), in1=rrt[:],
                                            s0=C23, s1=CL1, imm2=LC[4])
                      nc.vector._custom_dve(FUSE_LOG2, out=lam[:],
                                            in0=rrt[:], in1=e1t[:],
                                            s0=LC[3], s1=LC[2], imm2=LC[1])
                      # u = 1 + v*(EC0 + v*(EC1 + v*EC2)), v = max(lam*a/16, -1)
                      # (the /16 is folded into the PW constant tile)
                      nc.vector._custom_dve(FUSE_EXPA, out=u[:],
                                            in0=lam[:], in1=pw,
                                            s0=EC[0], s1=EC[1], imm2=EC[2])
                      # h = u^16 * Z = r^a * [pn | bn], written straight into
                      # the chunk output buffer (streamed out, summed on host)
                      nc.vector._custom_dve(FUSE_EXP3, out=h,
                                            in0=u[:], in1=z8)
                      # w = r1 * [A'1 | pc12] (r1 broadcast to both halves)
                      r1b = rrt[:, 0:4].rearrange('p (o f) -> p o f', o=1) \
                                       .broadcast_to([P, 2, 4])
                      a3 = a8.rearrange('p (o f) -> p o f', o=2)
                      w3 = w[:].rearrange('p (o f) -> p o f', o=2)
                      nc.vector.tensor_tensor(out=w3, in0=r1b, in1=a3, op=OP.mult)
                      # phi = [w_l + pn | w_r + r2]
                      nc.vector.tensor_tensor(out=phi[:, 0:4], in0=w[:, 0:4],
                                              in1=z8[:, 0:4], op=OP.add)
                      nc.vector.tensor_tensor(out=phi[:, 4:8], in0=w[:, 4:8],
                                              in1=rrt[:, 4:8], op=OP.add)
                      # r' = clip(phi - h, eps, 1)
                      nc.vector._custom_dve(FUSE_RCLIP, out=rrt[:],
                                            in0=phi[:], in1=h, s0=EPS)

                  nc.sync.dma_start(
                      out=HO[:, ds(ci * (k_chunk * 8), k_chunk * 8)],
                      in_=hc[:])
    nc.compile()
    return nc


def _host_prepare(forcing, initial_state, raw_params, param_lower, param_upper,
                  t_total=T):
    """Derive per-core input arrays. All fp32, same op order as the device."""
    f32 = np.float32
    lo = param_lower.astype(f32)
    hi = param_upper.astype(f32)
    sg = (1.0 / (1.0 + np.exp(-raw_params.astype(np.float64))))
    phys = (lo.astype(np.float64) + (hi - lo).astype(np.float64) * sg).astype(f32)
    mw1, mw2, percrte, baserte, qbp, axv = [phys[:, i].copy() for i in range(6)]
    inv1 = (f32(1.0) / mw1).astype(f32)
    inv2 = (f32(1.0) / mw2).astype(f32)

    p_r = forcing[:, :, 0].astype(f32)    # [T, H]
    pet = forcing[:, :, 1].astype(f32)

    pn = (p_r * inv1[None, :]).astype(f32)
    ap1 = (f32(1.0) - ((pet + percrte[None, :]) * inv1[None, :])).astype(f32)
    bn = (baserte * inv2).astype(f32)
    pc12 = (percrte * inv2).astype(f32)

    s1n = np.clip(initial_state[:, 0].astype(f32) * inv1, EPS, 1.0).astype(f32)
    s2n = np.clip(initial_state[:, 1].astype(f32) * inv2, EPS, 1.0).astype(f32)

    in_maps = []
    for c in range(NCORES):
        sl = slice(c * HC, (c + 1) * HC)
        # [T, HC] -> [T, G, P] -> [P, T, G]
        def tg(a):
            return np.ascontiguousarray(
                a[:, sl].reshape(t_total, G, P).transpose(2, 0, 1))
        ZA = np.empty((P, t_total, 16), f32)
        ZA[:, :, 0:4] = tg(pn)
        ZA[:, :, 4:8] = bn[sl].reshape(G, P).T[:, None, :]
        ZA[:, :, 8:12] = tg(ap1)
        ZA[:, :, 12:16] = pc12[sl].reshape(G, P).T[:, None, :]

        def pk(a1, a2):
            out = np.empty((P, 8), f32)
            out[:, 0:4] = a1[sl].reshape(G, P).T
            out[:, 4:8] = a2[sl].reshape(G, P).T
            return out

        # exponent tile pre-divided by 16 (the exp2 range reduction)
        consts = np.concatenate([pk(s1n, s2n),
                                 pk(axv / np.float32(16.0),
                                    qbp / np.float32(16.0)),
                                 pk(mw1, mw2)], axis=1)
        in_maps.append({
            "ZA": ZA.reshape(P, t_total * 16),
            "CONSTS": consts,
        })
    return in_maps, (mw1, mw2)


_NC_CACHE = {}


def kernel(forcing, initial_state, raw_params, param_lower, param_upper):
    forcing = np.asarray(forcing)
    initial_state = np.asarray(initial_state)
    raw_params = np.asarray(raw_params)
    param_lower = np.asarray(param_lower)
    param_upper = np.asarray(param_upper)
    t_total = forcing.shape[0]
    if t_total not in _NC_CACHE:
        _NC_CACHE[t_total] = build_nc(t_total=t_total)
    nc = _NC_CACHE[t_total]
    in_maps, (mw1, mw2) = _host_prepare(forcing, initial_state, raw_params,
                                        param_lower, param_upper,
                                        t_total=t_total)
    res = run_bass_kernel_spmd(nc, in_maps, core_ids=list(range(NCORES)))
    # per-core HO: [P, T, 8]; cols g / 4+g are (qsx/m1, qb/m2) of HRU g*P+p;
    # runoff = h1*m1 + h2*m2
    out = np.empty((t_total, H), np.float32)
    for c in range(NCORES):
        sl = slice(c * HC, (c + 1) * HC)
        ho = res.results[c]["HO"].reshape(P, t_total, 8)
        m1c = mw1[sl].reshape(G, P)
        m2c = mw2[sl].reshape(G, P)
        for g in range(G):
            h1 = ho[:, :, g]                # [P, T]
            h2 = ho[:, :, 4 + g]
            out[:, c * HC + g * P:c * HC + (g + 1) * P] = (
                h1 * m1c[g][:, None] + h2 * m2c[g][:, None]).T.astype(np.float32)
    return out


# revision 24
# speedup vs baseline: 1.7791x; 1.0438x over previous
"""FUSE bucket-model scan kernel for Trainium2 (8 NeuronCores) — all-DVE scan.

Strategy
--------
H=4096 HRUs sharded across 8 cores (512 each) as [128 partitions x 4 groups];
the two bucket states are packed into one [128, 8] tile (cols 0-3: upper zone
per group, cols 4-7: lower zone). The T=8192 time recurrence is a sequential
scan; all forcing-derived operand tiles (Z = [pn | bn], A' = [1-(pet+perc)/m1
| perc/m2]) are precomputed on the host in a scan-friendly [128, T, 16]
layout and streamed per K-step chunk.

The baseline ran the power nonlinearity (r^a = exp(a ln r)) through the
Activation engine: two transcendental round-trips + cross-engine hops gave a
~1.5us/step loop-carried chain. This kernel instead computes r^a entirely on
the Vector engine (DVE) with fused custom multi-stage ops, so every op in the
cycle sits on ONE engine queue and dependent ops chain back-to-back at
~70ns/instruction with no semaphore hops:

  LOG1/LOG2: log2(r) via IEEE-754 bit tricks — mu = bits(r)|bits(1.0) in
    [1,2) (1 ALU stage; valid for r<2), exponent from float(int32-view)*2^-23,
    deg-4 minimax poly for log2(mu);
  EXP1: y = log2(r)*a (per-element exponents), v = max(y/16, -1);
  EXP2/EXP3: 2^y = ((1 + v*P2(v))^16) * Z — constrained deg-3 poly + 4
    squarings, with the h = x*Z multiply folded into the last op;
  W/PHI: phi = [r1*A'1 + pn | r1*pc12 + r2] (stock tensor ops);
  RCLIP: r' = clip(phi - h, eps, 1) (fused clip-sub custom op).

9 DVE instructions/step total; runoff (h*MW pair-summed) and the chunk
stream pre-touch run on Pool in the latency shadow. State is normalized
r = clip(s/maxwatr, eps, 1); carrying r instead of (sigma, r) loses at most
eps/step, far under tolerance. End-to-end vs the f64 reference: rel-of-max
~3e-4 (tolerance 2e-2). Model-estimated device time ~5ms vs 12.3ms baseline.
"""
import numpy as np

import concourse.bass as bass
import concourse.bacc as bacc
import concourse.mybir as mybir
from concourse.bass import ds
from concourse.tile import TileContext
from concourse.bass_utils import run_bass_kernel_spmd

F32 = mybir.dt.float32
I32 = mybir.dt.int32
AF = mybir.ActivationFunctionType
OP = mybir.AluOpType

# --- custom fused DVE ops ---------------------------------------------------
from concourse.dve_spec import Spec, Src0, Src1, maxx, minn, Idx, lower as _dve_lower
from concourse.dve_spec import C0 as _C0, C1 as _C1, C2 as _C2, One as _One, Zero as _Zero
from concourse.dve_spec import Bin, AluOp
from concourse import dve_ops as _dvo
from concourse.dve_uop import DveOpSpec as _DveOpSpec


def _register_custom_op(name, spec):
    for op in _dvo.OPS:
        if op.name == name:
            return op
    row = _dvo._CUSTOM_DVE_ROW_BASE + len(_dvo.OPS)
    _dvo._SUB_OPCODE_FOR_NAME[name] = row
    shas = {}
    for ver in ("v3", "v4"):
        try:
            uops = _dve_lower(spec, ver=ver)
            shas[ver] = _DveOpSpec(name=name, opcode=row, uops=uops,
                                   rd1_en=True).sha(ver)
        except Exception:
            pass
    op = _dvo.DveOp(name, spec, subdim=False, uops_sha=shas)
    _dvo.OPS.append(op)
    _dvo.CUSTOM_DVE_SPECS[name] = spec
    return op


# Polynomial coefficients (Chebyshev LS fits).
# log2(mu) on [1,2], deg 4 (c0 folded into the LOG1 bias constant):
LC = [-2.496773767905599, 4.0283727668469735, -2.0810602034595114,
      0.6288157291849531, -0.07915036575317282]
# (2^v - 1)/v on [-1,0], deg 2  =>  2^v ~= 1 + v*(EC0 + v*(EC1 + v*EC2)):
EC = [0.6927658142758559, 0.23552592911390743, 0.043112826547017494]
C23 = float(2.0 ** -23)
CL1 = float(126.0 - LC[0])

_f = lambda x: np.asarray(x, np.float32)


def _mu_of(in1):
    return (np.asarray(in1, np.float32).view(np.int32)
            | np.int32(0x3F800000)).view(np.float32)


def _ref_log1(in0, in1, s0, s1, imm2):
    f = in0.astype(np.float32)
    y0 = _f(_f(f * np.float32(s0)) - np.float32(s1))
    mu = _mu_of(in1)
    e1 = _f(y0 - mu)
    mu2 = _f(mu * mu)
    mu4 = _f(mu2 * mu2)
    return _f(e1 + _f(mu4 * np.float32(imm2)))


_mu0 = Bin(AluOp.BITWISE_OR, Src1, _One)
_mu0sq = _mu0 * _mu0
FUSE_LOG1 = _register_custom_op(
    "FUSE_LOG1",
    Spec(body=((Src0 * _C0 - _C1) - _mu0) + (_mu0sq * _mu0sq) * _C2,
         reference=_ref_log1),
)


def _ref_log2(in0, in1, s0, s1, imm2):
    mu = _mu_of(in0)
    E = _f(_f(_f(_f(_f(mu * np.float32(s0)) + np.float32(s1)) * mu)
              + np.float32(imm2)) * mu)
    return _f(E + in1.astype(np.float32))


_mu1 = Bin(AluOp.BITWISE_OR, Src0, _One)
FUSE_LOG2 = _register_custom_op(
    "FUSE_LOG2",
    Spec(body=((_mu1 * _C0 + _C1) * _mu1 + _C2) * _mu1 + Src1,
         reference=_ref_log2),
)


def _ref_expa(in0, in1, s0, s1, imm2):
    # in0 = log2(r), in1 = PW/16 (range reduction host-folded into the
    # exponent constant tile)
    y = _f(in0.astype(np.float32) * in1.astype(np.float32))
    v = np.maximum(y, np.float32(-1.0))
    q = _f(_f(_f(_f(_f(v * np.float32(imm2)) + np.float32(s1)) * v)
              + np.float32(s0)) * v)
    return _f(q + np.float32(1.0))


_v = maxx(Src0 * Src1, _Zero - _One)
_q = ((_v * _C2 + _C1) * _v + _C0) * _v
FUSE_EXPA = _register_custom_op(
    "FUSE_EXPA",
    Spec(body=_q + _One, reference=_ref_expa),
)


def _ref_exp3b(in0, in1, s0, s1, imm2):
    # hD = (beta - u^16) * Z, beta = 1 for cols < s0 (upper zone), else 0:
    # left half gives pn*(1 - x1), right half gives -bn*x2.
    u = in0.astype(np.float32)
    u2 = _f(u * u)
    u4 = _f(u2 * u2)
    u8 = _f(u4 * u4)
    u16 = _f(u8 * u8)
    n = in0.shape[-1]
    beta = (np.arange(n, dtype=np.float32) < np.float32(s0)).astype(np.float32)
    return _f(_f(beta - u16) * in1.astype(np.float32))


_su2 = Src0 * Src0
_su4 = _su2 * _su2
_su8 = _su4 * _su4
_su16 = _su8 * _su8
FUSE_EXP3B = _register_custom_op(
    "FUSE_EXP3B",
    Spec(body=((Idx < _C0) - _su16) * Src1, reference=_ref_exp3b),
)


def _ref_clipa(in0, in1, s0, s1, imm2):
    return np.clip(_f(in0.astype(np.float32) + in1.astype(np.float32)),
                   np.float32(s0), np.float32(1.0))


FUSE_RCLIPA = _register_custom_op(
    "FUSE_RCLIPA",
    Spec(body=minn(maxx(Src0 + Src1, _C0), _One), reference=_ref_clipa),
)

T = 8192
H = 4096
NCORES = 8
HC = H // NCORES          # 512 HRUs per core
P = 128                   # partitions
G = HC // P               # 4 groups
K = 128                   # timesteps per chunk
EPS = 1e-6


def build_nc(t_total=T, k_chunk=K, unrolled=False):
    nc = bacc.Bacc()
    ZAt = nc.dram_tensor("ZA", [P, t_total * 16], F32, kind="ExternalInput")
    Ct = nc.dram_tensor("CONSTS", [P, 24], F32, kind="ExternalInput")
    # raw h = r^a * [pn | bn] streamed out; host folds maxwatr + pair-sum
    HO = nc.dram_tensor("HO", [P, t_total * 8], F32, kind="ExternalOutput")

    n_chunks = t_total // k_chunk
    with TileContext(nc) as tc:
        with (
            tc.tile_pool(name="const", bufs=1) as cpool,
            tc.tile_pool(name="zin", bufs=3) as zpool,
            tc.tile_pool(name="rout", bufs=3) as ropool,
            tc.tile_pool(name="work", bufs=4) as wpool,
        ):
            cst_in = cpool.tile([P, 24], F32)
            cst = cpool.tile([P, 24], F32)
            nc.sync.dma_start(out=cst_in[:], in_=Ct[:])
            nc.vector.tensor_copy(out=cst[:], in_=cst_in[:])
            pw = cst[:, 8:16]
            rrt = cpool.tile([P, 8], F32)
            # state: r = clip(s/maxwatr, eps, 1); all scan ops live on DVE
            nc.vector.tensor_scalar(out=rrt[:], in0=cst[:, 0:8], scalar1=EPS,
                                    scalar2=None, op0=OP.max)

            import contextlib
            def chunk_iter():
                if unrolled:
                    for i in range(n_chunks):
                        yield contextlib.nullcontext(i)
                else:
                    yield tc.For_i(0, n_chunks, staggered_reset=True,
                                   hint_engines=(mybir.EngineType.DVE,
                                                 mybir.EngineType.Pool))
            for _cm in chunk_iter():
              with _cm as ci:
                  zc = zpool.tile([P, k_chunk * 16], F32)
                  hc = ropool.tile([P, k_chunk * 8], F32)
                  nc.sync.dma_start(out=zc[:], in_=ZAt[:, ds(ci * (k_chunk * 16), k_chunk * 16)])
                  # pre-touch on the otherwise-idle Activation engine: absorbs
                  # the DMA-completion wait so per-step stream readers only
                  # ever wait on one compute semaphore.
                  zc2 = zpool.tile([P, k_chunk * 16], F32, tag="zc2")
                  nc.scalar.activation(zc2[:], zc[:], AF.Copy)

                  for k in range(k_chunk):
                      z8 = zc2[:, k * 16:k * 16 + 8]
                      a8 = zc2[:, k * 16 + 8:k * 16 + 16]
                      e1t = wpool.tile([P, 8], F32, tag="e1t")
                      lam = wpool.tile([P, 8], F32, tag="lam")
                      u = wpool.tile([P, 8], F32, tag="u")
                      h = hc[:, k * 8:k * 8 + 8]
                      phi = wpool.tile([P, 8], F32, tag="phi")

                      # 5-node dependent DVE chain (~164ns/hop); the two
                      # fillers (W, PHIR) are placed so each lands in one
                      # result-drain gap and nothing delays a chain node.
                      # log2(r) = e' + P4(mu) via int32-view exponent + OR-mantissa
                      nc.vector._custom_dve(FUSE_LOG1, out=e1t[:],
                                            in0=rrt[:].bitcast(I32), in1=rrt[:],
                                            s0=C23, s1=CL1, imm2=LC[4])
                      # filler 1: phi = r1 * [A'1 | pc12] (r1 broadcast)
                      r1b = rrt[:, 0:4].rearrange('p (o f) -> p o f', o=1) \
                                       .broadcast_to([P, 2, 4])
                      a3 = a8.rearrange('p (o f) -> p o f', o=2)
                      w3 = phi[:].rearrange('p (o f) -> p o f', o=2)
                      nc.vector.tensor_tensor(out=w3, in0=r1b, in1=a3, op=OP.mult)
                      nc.vector._custom_dve(FUSE_LOG2, out=lam[:],
                                            in0=rrt[:], in1=e1t[:],
                                            s0=LC[3], s1=LC[2], imm2=LC[1])
                      # filler 2: phi_r += r2 (in place)
                      nc.vector.tensor_tensor(out=phi[:, 4:8], in0=phi[:, 4:8],
                                              in1=rrt[:, 4:8], op=OP.add)
                      # u = 1 + v*(EC0 + v*(EC1 + v*EC2)), v = max(lam*a/16, -1)
                      # (the /16 is folded into the PW constant tile)
                      nc.vector._custom_dve(FUSE_EXPA, out=u[:],
                                            in0=lam[:], in1=pw,
                                            s0=EC[0], s1=EC[1], imm2=EC[2])
                      # hD = ((col<4) - u^16) * Z = [pn*(1-x1) | -bn*x2],
                      # written straight into the chunk output buffer
                      # (streamed out; host turns it into runoff)
                      nc.vector._custom_dve(FUSE_EXP3B, out=h,
                                            in0=u[:], in1=z8, s0=4.0)
                      # r' = clip(phi + hD, eps, 1)
                      nc.vector._custom_dve(FUSE_RCLIPA, out=rrt[:],
                                            in0=phi[:], in1=h, s0=EPS)

                  nc.sync.dma_start(
                      out=HO[:, ds(ci * (k_chunk * 8), k_chunk * 8)],
                      in_=hc[:])
    nc.compile()
    return nc


def _host_prepare(forcing, initial_state, raw_params, param_lower, param_upper,
                  t_total=T):
    """Derive per-core input arrays. All fp32, same op order as the device."""
    f32 = np.float32
    lo = param_lower.astype(f32)
    hi = param_upper.astype(f32)
    sg = (1.0 / (1.0 + np.exp(-raw_params.astype(np.float64))))
    phys = (lo.astype(np.float64) + (hi - lo).astype(np.float64) * sg).astype(f32)
    mw1, mw2, percrte, baserte, qbp, axv = [phys[:, i].copy() for i in range(6)]
    inv1 = (f32(1.0) / mw1).astype(f32)
    inv2 = (f32(1.0) / mw2).astype(f32)

    p_r = forcing[:, :, 0].astype(f32)    # [T, H]
    pet = forcing[:, :, 1].astype(f32)

    pn = (p_r * inv1[None, :]).astype(f32)
    ap1 = (f32(1.0) - ((pet + percrte[None, :]) * inv1[None, :])).astype(f32)
    bn = (baserte * inv2).astype(f32)
    pc12 = (percrte * inv2).astype(f32)

    s1n = np.clip(initial_state[:, 0].astype(f32) * inv1, EPS, 1.0).astype(f32)
    s2n = np.clip(initial_state[:, 1].astype(f32) * inv2, EPS, 1.0).astype(f32)

    in_maps = []
    for c in range(NCORES):
        sl = slice(c * HC, (c + 1) * HC)
        # [T, HC] -> [T, G, P] -> [P, T, G]
        def tg(a):
            return np.ascontiguousarray(
                a[:, sl].reshape(t_total, G, P).transpose(2, 0, 1))
        ZA = np.empty((P, t_total, 16), f32)
        ZA[:, :, 0:4] = tg(pn)
        ZA[:, :, 4:8] = bn[sl].reshape(G, P).T[:, None, :]
        ZA[:, :, 8:12] = tg(ap1)
        ZA[:, :, 12:16] = pc12[sl].reshape(G, P).T[:, None, :]

        def pk(a1, a2):
            out = np.empty((P, 8), f32)
            out[:, 0:4] = a1[sl].reshape(G, P).T
            out[:, 4:8] = a2[sl].reshape(G, P).T
            return out

        # exponent tile pre-divided by 16 (the exp2 range reduction)
        consts = np.concatenate([pk(s1n, s2n),
                                 pk(axv / np.float32(16.0),
                                    qbp / np.float32(16.0)),
                                 pk(mw1, mw2)], axis=1)
        in_maps.append({
            "ZA": ZA.reshape(P, t_total * 16),
            "CONSTS": consts,
        })
    return in_maps, (mw1, mw2)


_NC_CACHE = {}


def kernel(forcing, initial_state, raw_params, param_lower, param_upper):
    forcing = np.asarray(forcing)
    initial_state = np.asarray(initial_state)
    raw_params = np.asarray(raw_params)
    param_lower = np.asarray(param_lower)
    param_upper = np.asarray(param_upper)
    t_total = forcing.shape[0]
    if t_total not in _NC_CACHE:
        _NC_CACHE[t_total] = build_nc(t_total=t_total)
    nc = _NC_CACHE[t_total]
    in_maps, (mw1, mw2) = _host_prepare(forcing, initial_state, raw_params,
                                        param_lower, param_upper,
                                        t_total=t_total)
    res = run_bass_kernel_spmd(nc, in_maps, core_ids=list(range(NCORES)))
    # per-core HO: [P, T, 8]; cols g / 4+g hold hD = (pn*(1-x1), -bn*x2) of
    # HRU g*P+p; runoff = qsx + qb = p - (hD1*m1 + hD2*m2)
    p_raw = forcing[:, :, 0].astype(np.float32)    # [T, H]
    out = np.empty((t_total, H), np.float32)
    for c in range(NCORES):
        sl = slice(c * HC, (c + 1) * HC)
        ho = res.results[c]["HO"].reshape(P, t_total, 8)
        m1c = mw1[sl].reshape(G, P)
        m2c = mw2[sl].reshape(G, P)
        for g in range(G):
            hd1 = ho[:, :, g]                # [P, T]
            hd2 = ho[:, :, 4 + g]
            cols = slice(c * HC + g * P, c * HC + (g + 1) * P)
            out[:, cols] = p_raw[:, cols] - (
                hd1 * m1c[g][:, None] + hd2 * m2c[g][:, None]).T
    return out


# revision 28
# speedup vs baseline: 2.0043x; 1.1266x over previous
"""FUSE bucket-model scan kernel for Trainium2 (8 NeuronCores) — all-DVE scan.

Strategy
--------
H=4096 HRUs sharded across 8 cores (512 each) as [128 partitions x 4 groups];
the two bucket states are packed into one [128, 8] tile (cols 0-3: upper zone
per group, cols 4-7: lower zone). The T=8192 time recurrence is a sequential
scan; all forcing-derived operand tiles (Z = [pn | bn], A' = [1-(pet+perc)/m1
| perc/m2]) are precomputed on the host in a scan-friendly [128, T, 16]
layout and streamed per K-step chunk.

The baseline ran the power nonlinearity (r^a = exp(a ln r)) through the
Activation engine: two transcendental round-trips + cross-engine hops gave a
~1.5us/step loop-carried chain. This kernel instead computes r^a entirely on
the Vector engine (DVE) with fused custom multi-stage ops, so every op in the
cycle sits on ONE engine queue and dependent ops chain back-to-back at
~70ns/instruction with no semaphore hops:

  LOG1/LOG2: log2(r) via IEEE-754 bit tricks — mu = bits(r)|bits(1.0) in
    [1,2) (1 ALU stage; valid for r<2), exponent from float(int32-view)*2^-23,
    deg-4 minimax poly for log2(mu);
  EXP1: y = log2(r)*a (per-element exponents), v = max(y/16, -1);
  EXP2/EXP3: 2^y = ((1 + v*P2(v))^16) * Z — constrained deg-3 poly + 4
    squarings, with the h = x*Z multiply folded into the last op;
  W/PHI: phi = [r1*A'1 + pn | r1*pc12 + r2] (stock tensor ops);
  RCLIP: r' = clip(phi - h, eps, 1) (fused clip-sub custom op).

9 DVE instructions/step total; runoff (h*MW pair-summed) and the chunk
stream pre-touch run on Pool in the latency shadow. State is normalized
r = clip(s/maxwatr, eps, 1); carrying r instead of (sigma, r) loses at most
eps/step, far under tolerance. End-to-end vs the f64 reference: rel-of-max
~3e-4 (tolerance 2e-2). Model-estimated device time ~5ms vs 12.3ms baseline.
"""
import numpy as np

import concourse.bass as bass
import concourse.bacc as bacc
import concourse.mybir as mybir
from concourse.bass import ds
from concourse.tile import TileContext
from concourse.bass_utils import run_bass_kernel_spmd

F32 = mybir.dt.float32
I32 = mybir.dt.int32
AF = mybir.ActivationFunctionType
OP = mybir.AluOpType

# --- custom fused DVE ops ---------------------------------------------------
from concourse.dve_spec import Spec, Src0, Src1, maxx, minn, Idx, lower as _dve_lower
from concourse.dve_spec import C0 as _C0, C1 as _C1, C2 as _C2, One as _One, Zero as _Zero
from concourse.dve_spec import Bin, AluOp
from concourse import dve_ops as _dvo
from concourse.dve_uop import DveOpSpec as _DveOpSpec


def _register_custom_op(name, spec):
    for op in _dvo.OPS:
        if op.name == name:
            return op
    row = _dvo._CUSTOM_DVE_ROW_BASE + len(_dvo.OPS)
    _dvo._SUB_OPCODE_FOR_NAME[name] = row
    shas = {}
    for ver in ("v3", "v4"):
        try:
            uops = _dve_lower(spec, ver=ver)
            shas[ver] = _DveOpSpec(name=name, opcode=row, uops=uops,
                                   rd1_en=True).sha(ver)
        except Exception:
            pass
    op = _dvo.DveOp(name, spec, subdim=False, uops_sha=shas)
    _dvo.OPS.append(op)
    _dvo.CUSTOM_DVE_SPECS[name] = spec
    return op


# Polynomial coefficients (Chebyshev LS fits).
# log2(mu) on [1,2], deg 3 (c0 folded into the LOGE bias constant):
LC = [-2.1338165360123584, 3.0107302949770647, -1.0294927543909727,
      0.15391353466591032]
# (2^v - 1)/v on [-1,0], deg 2  =>  2^v ~= 1 + v*(EC0 + v*(EC1 + v*EC2)):
EC = [0.6927658142758559, 0.23552592911390743, 0.043112826547017494]
C23 = float(2.0 ** -23)
CL1 = float(126.0 - LC[0])

_f = lambda x: np.asarray(x, np.float32)


def _mu_of(in1):
    return (np.asarray(in1, np.float32).view(np.int32)
            | np.int32(0x3F800000)).view(np.float32)


def _ref_loge(in0, in1, s0, s1, imm2):
    # in0 = int32 view of r; in1 = PW/16. out = (float(Iv)*2^-23 - CL1)*a/16
    f = in0.astype(np.float32)
    y0 = _f(_f(f * np.float32(s0)) - np.float32(s1))
    return _f(y0 * in1.astype(np.float32))


FUSE_LOGE = _register_custom_op(
    "FUSE_LOGE",
    Spec(body=(Src0 * _C0 - _C1) * Src1, reference=_ref_loge),
)


def _ref_logp(in0, in1, s0, s1, imm2):
    # in0 = r (f32); in1 = PW/16. out = (P3(mu) - mu)*a/16
    mu = _mu_of(in0)
    E = _f(_f(_f(_f(_f(mu * np.float32(s0)) + np.float32(s1)) * mu)
              + np.float32(imm2)) * mu)
    return _f(_f(E - mu) * in1.astype(np.float32))


_mu1 = Bin(AluOp.BITWISE_OR, Src0, _One)
FUSE_LOGP = _register_custom_op(
    "FUSE_LOGP",
    Spec(body=((((_mu1 * _C0 + _C1) * _mu1 + _C2) * _mu1) - _mu1) * Src1,
         reference=_ref_logp),
)


def _ref_expb(in0, in1, s0, s1, imm2):
    # v = max(loge_part + logp_part, -1); u = 1 + v*(EC0 + v*(EC1 + v*EC2))
    y = _f(in0.astype(np.float32) + in1.astype(np.float32))
    v = np.maximum(y, np.float32(-1.0))
    q = _f(_f(_f(_f(_f(v * np.float32(imm2)) + np.float32(s1)) * v)
              + np.float32(s0)) * v)
    return _f(q + np.float32(1.0))


_v = maxx(Src0 + Src1, _Zero - _One)
_q = ((_v * _C2 + _C1) * _v + _C0) * _v
FUSE_EXPB = _register_custom_op(
    "FUSE_EXPB",
    Spec(body=_q + _One, reference=_ref_expb),
)


def _ref_exp3b(in0, in1, s0, s1, imm2):
    # hD = (beta - u^16) * Z, beta = 1 for cols < s0 (upper zone), else 0:
    # left half gives pn*(1 - x1), right half gives -bn*x2.
    u = in0.astype(np.float32)
    u2 = _f(u * u)
    u4 = _f(u2 * u2)
    u8 = _f(u4 * u4)
    u16 = _f(u8 * u8)
    n = in0.shape[-1]
    beta = (np.arange(n, dtype=np.float32) < np.float32(s0)).astype(np.float32)
    return _f(_f(beta - u16) * in1.astype(np.float32))


_su2 = Src0 * Src0
_su4 = _su2 * _su2
_su8 = _su4 * _su4
_su16 = _su8 * _su8
FUSE_EXP3B = _register_custom_op(
    "FUSE_EXP3B",
    Spec(body=((Idx < _C0) - _su16) * Src1, reference=_ref_exp3b),
)


def _ref_clipa(in0, in1, s0, s1, imm2):
    return np.clip(_f(in0.astype(np.float32) + in1.astype(np.float32)),
                   np.float32(s0), np.float32(1.0))


FUSE_RCLIPA = _register_custom_op(
    "FUSE_RCLIPA",
    Spec(body=minn(maxx(Src0 + Src1, _C0), _One), reference=_ref_clipa),
)

T = 8192
H = 4096
NCORES = 8
HC = H // NCORES          # 512 HRUs per core
P = 128                   # partitions
G = HC // P               # 4 groups
K = 128                   # timesteps per chunk
EPS = 1e-6


def build_nc(t_total=T, k_chunk=K, unrolled=False):
    nc = bacc.Bacc()
    ZAt = nc.dram_tensor("ZA", [P, t_total * 16], F32, kind="ExternalInput")
    Ct = nc.dram_tensor("CONSTS", [P, 24], F32, kind="ExternalInput")
    # raw h = r^a * [pn | bn] streamed out; host folds maxwatr + pair-sum
    HO = nc.dram_tensor("HO", [P, t_total * 8], F32, kind="ExternalOutput")

    n_chunks = t_total // k_chunk
    with TileContext(nc) as tc:
        with (
            tc.tile_pool(name="const", bufs=1) as cpool,
            tc.tile_pool(name="zin", bufs=3) as zpool,
            tc.tile_pool(name="rout", bufs=3) as ropool,
            tc.tile_pool(name="work", bufs=4) as wpool,
        ):
            cst_in = cpool.tile([P, 24], F32)
            cst = cpool.tile([P, 24], F32)
            nc.sync.dma_start(out=cst_in[:], in_=Ct[:])
            nc.vector.tensor_copy(out=cst[:], in_=cst_in[:])
            pw = cst[:, 8:16]
            rrt = cpool.tile([P, 8], F32)
            # state: r = clip(s/maxwatr, eps, 1); all scan ops live on DVE
            nc.vector.tensor_scalar(out=rrt[:], in0=cst[:, 0:8], scalar1=EPS,
                                    scalar2=None, op0=OP.max)

            import contextlib
            def chunk_iter():
                if unrolled:
                    for i in range(n_chunks):
                        yield contextlib.nullcontext(i)
                else:
                    yield tc.For_i(0, n_chunks, staggered_reset=True,
                                   hint_engines=(mybir.EngineType.DVE,
                                                 mybir.EngineType.Pool))
            for _cm in chunk_iter():
              with _cm as ci:
                  zc = zpool.tile([P, k_chunk * 16], F32)
                  hc = ropool.tile([P, k_chunk * 8], F32)
                  nc.sync.dma_start(out=zc[:], in_=ZAt[:, ds(ci * (k_chunk * 16), k_chunk * 16)])
                  # pre-touch on the otherwise-idle Activation engine: absorbs
                  # the DMA-completion wait so per-step stream readers only
                  # ever wait on one compute semaphore.
                  zc2 = zpool.tile([P, k_chunk * 16], F32, tag="zc2")
                  nc.scalar.activation(zc2[:], zc[:], AF.Copy)

                  for k in range(k_chunk):
                      z8 = zc2[:, k * 16:k * 16 + 8]
                      a8 = zc2[:, k * 16 + 8:k * 16 + 16]
                      et = wpool.tile([P, 8], F32, tag="et")
                      pt = wpool.tile([P, 8], F32, tag="pt")
                      u = wpool.tile([P, 8], F32, tag="u")
                      h = hc[:, k * 8:k * 8 + 8]
                      phi = wpool.tile([P, 8], F32, tag="phi")

                      # 4-hop dependent DVE chain (~164ns/hop): the two log
                      # halves both depend only on r, so they run back-to-back
                      # at the cycle start and EXPB merges them; W and PHIR
                      # fill the result-drain gaps.
                      # exponent half: (float(int32(r))*2^-23 - CL1) * a/16
                      nc.vector._custom_dve(FUSE_LOGE, out=et[:],
                                            in0=rrt[:].bitcast(I32), in1=pw,
                                            s0=C23, s1=CL1)
                      # mantissa half: (P3(mu) - mu) * a/16, mu = bits(r)|1.0
                      nc.vector._custom_dve(FUSE_LOGP, out=pt[:],
                                            in0=rrt[:], in1=pw,
                                            s0=LC[3], s1=LC[2], imm2=LC[1])
                      # filler 1: phi = r1 * [A'1 | pc12] (r1 broadcast)
                      r1b = rrt[:, 0:4].rearrange('p (o f) -> p o f', o=1) \
                                       .broadcast_to([P, 2, 4])
                      a3 = a8.rearrange('p (o f) -> p o f', o=2)
                      w3 = phi[:].rearrange('p (o f) -> p o f', o=2)
                      nc.vector.tensor_tensor(out=w3, in0=r1b, in1=a3, op=OP.mult)
                      # filler 2: phi_r += r2 (in place)
                      nc.vector.tensor_tensor(out=phi[:, 4:8], in0=phi[:, 4:8],
                                              in1=rrt[:, 4:8], op=OP.add)
                      # u = 1 + v*(EC0 + v*(EC1 + v*EC2)), v = max(et+pt, -1)
                      nc.vector._custom_dve(FUSE_EXPB, out=u[:],
                                            in0=et[:], in1=pt[:],
                                            s0=EC[0], s1=EC[1], imm2=EC[2])
                      # hD = ((col<4) - u^16) * Z = [pn*(1-x1) | -bn*x2],
                      # written straight into the chunk output buffer
                      # (streamed out; host turns it into runoff)
                      nc.vector._custom_dve(FUSE_EXP3B, out=h,
                                            in0=u[:], in1=z8, s0=4.0)
                      # r' = clip(phi + hD, eps, 1)
                      nc.vector._custom_dve(FUSE_RCLIPA, out=rrt[:],
                                            in0=phi[:], in1=h, s0=EPS)

                  # output DMA issued from the (otherwise idle) Pool queue so
                  # its long wait on hc never blocks SP's input prefetch
                  nc.gpsimd.dma_start(
                      out=HO[:, ds(ci * (k_chunk * 8), k_chunk * 8)],
                      in_=hc[:])
    nc.compile()
    return nc


def _host_prepare(forcing, initial_state, raw_params, param_lower, param_upper,
                  t_total=T):
    """Derive per-core input arrays. All fp32, same op order as the device."""
    f32 = np.float32
    lo = param_lower.astype(f32)
    hi = param_upper.astype(f32)
    sg = (1.0 / (1.0 + np.exp(-raw_params.astype(np.float64))))
    phys = (lo.astype(np.float64) + (hi - lo).astype(np.float64) * sg).astype(f32)
    mw1, mw2, percrte, baserte, qbp, axv = [phys[:, i].copy() for i in range(6)]
    inv1 = (f32(1.0) / mw1).astype(f32)
    inv2 = (f32(1.0) / mw2).astype(f32)

    p_r = forcing[:, :, 0].astype(f32)    # [T, H]
    pet = forcing[:, :, 1].astype(f32)

    pn = (p_r * inv1[None, :]).astype(f32)
    ap1 = (f32(1.0) - ((pet + percrte[None, :]) * inv1[None, :])).astype(f32)
    bn = (baserte * inv2).astype(f32)
    pc12 = (percrte * inv2).astype(f32)

    s1n = np.clip(initial_state[:, 0].astype(f32) * inv1, EPS, 1.0).astype(f32)
    s2n = np.clip(initial_state[:, 1].astype(f32) * inv2, EPS, 1.0).astype(f32)

    in_maps = []
    for c in range(NCORES):
        sl = slice(c * HC, (c + 1) * HC)
        # [T, HC] -> [T, G, P] -> [P, T, G]
        def tg(a):
            return np.ascontiguousarray(
                a[:, sl].reshape(t_total, G, P).transpose(2, 0, 1))
        ZA = np.empty((P, t_total, 16), f32)
        ZA[:, :, 0:4] = tg(pn)
        ZA[:, :, 4:8] = bn[sl].reshape(G, P).T[:, None, :]
        ZA[:, :, 8:12] = tg(ap1)
        ZA[:, :, 12:16] = pc12[sl].reshape(G, P).T[:, None, :]

        def pk(a1, a2):
            out = np.empty((P, 8), f32)
            out[:, 0:4] = a1[sl].reshape(G, P).T
            out[:, 4:8] = a2[sl].reshape(G, P).T
            return out

        # exponent tile pre-divided by 16 (the exp2 range reduction)
        consts = np.concatenate([pk(s1n, s2n),
                                 pk(axv / np.float32(16.0),
                                    qbp / np.float32(16.0)),
                                 pk(mw1, mw2)], axis=1)
        in_maps.append({
            "ZA": ZA.reshape(P, t_total * 16),
            "CONSTS": consts,
        })
    return in_maps, (mw1, mw2)


_NC_CACHE = {}


def kernel(forcing, initial_state, raw_params, param_lower, param_upper):
    forcing = np.asarray(forcing)
    initial_state = np.asarray(initial_state)
    raw_params = np.asarray(raw_params)
    param_lower = np.asarray(param_lower)
    param_upper = np.asarray(param_upper)
    t_total = forcing.shape[0]
    if t_total not in _NC_CACHE:
        _NC_CACHE[t_total] = build_nc(t_total=t_total)
    nc = _NC_CACHE[t_total]
    in_maps, (mw1, mw2) = _host_prepare(forcing, initial_state, raw_params,
                                        param_lower, param_upper,
                                        t_total=t_total)
    res = run_bass_kernel_spmd(nc, in_maps, core_ids=list(range(NCORES)))
    # per-core HO: [P, T, 8]; cols g / 4+g hold hD = (pn*(1-x1), -bn*x2) of
    # HRU g*P+p; runoff = qsx + qb = p - (hD1*m1 + hD2*m2)
    p_raw = forcing[:, :, 0].astype(np.float32)    # [T, H]
    out = np.empty((t_total, H), np.float32)
    for c in range(NCORES):
        sl = slice(c * HC, (c + 1) * HC)
        ho = res.results[c]["HO"].reshape(P, t_total, 8)
        m1c = mw1[sl].reshape(G, P)
        m2c = mw2[sl].reshape(G, P)
        for g in range(G):
            hd1 = ho[:, :, g]                # [P, T]
            hd2 = ho[:, :, 4 + g]
            cols = slice(c * HC + g * P, c * HC + (g + 1) * P)
            out[:, cols] = p_raw[:, cols] - (
                hd1 * m1c[g][:, None] + hd2 * m2c[g][:, None]).T
    return out
